# revision 14
# baseline (speedup 1.0000x reference)
"""Trainium2 Bass kernel for APNet2 dAPNet2 MPNN readout + segment reduce.

Computes, for 500k atom pairs:
    E = MLP(hAB) + MLP(hBA)          (4-layer MLP, shared weights)
    delta_E = E * cutoff
    out = segment_sum(delta_E, dimer_ind, ndimer=2048)   -> [2048, 1]

Strategy v4 (8 NeuronCores, data-parallel over pairs):
  - Device computes only L1 -> L2 -> L3 (bf16 matmuls, fp32 psum) and
    streams the post-relu L3 activations (t3, bf16) straight to DRAM via
    DMA.  The tiny L4 (64->1 dot) and the segment reduce run on the HOST
    in fp32: they are ~0.2% of the FLOPs but on-device they cost a
    feature->pair transpose (one stationary load per 128 pairs) plus two
    extra PSUM evacuations, and the PSUM evacuation engines (ACT 1.2GHz
    + DVE 0.96GHz, 1 elem/lane/cycle, the only engines that can read
    PSUM) are co-bottlenecks of this kernel alongside the PE.
  - This also removes the host-side sort + one-hot segment machinery of
    the previous version: the host just does E = w4.T @ t3 and one
    np.bincount scatter over the unsorted pairs.
  - fp8 DoubleRow matmuls were evaluated and rejected: walrus's ISA
    check (s3d3_mm_valid_dst_partition) forbids DoubleRow outputs on
    partitions >= 64, so every DR output collapses onto partitions 0-63
    and all downstream activations become 64-partition tensors, doubling
    their PSUM-evacuation free-size.  The evacuation engines, not the
    PE, set the floor, so 2x matmul rate is a net loss.
  - Per 512-pair tile (1024 MLP rows): PE = 2048 (L1) + 2048 (L2) +
    1024 (L3) cycles @2.4GHz = 2.13us; ACT = y1 evac (2048 lanecycles);
    DVE = y2 + t3 evac (1536 lanecycles); DMA = 256KB x in + 128KB t3
    out.  All engines land just around ~2us/tile.
"""

import numpy as np
import ml_dtypes

BF = ml_dtypes.bfloat16
E3 = ml_dtypes.float8_e3m4

NCORES = 8
N_PAIRS = 500_000
FEAT = 128
NDIMER = 2048
H1, H2, H3 = 256, 128, 64
TILE = 512          # pairs per tile

# per-core pair count (padded)
PC = 62976          # = 512 * 123 ; 8 * 62976 = 503808 >= 500000
NT = PC // TILE     # 123


def _pick_gt(nt):
    for g in (4, 3, 2):
        if nt % g == 0:
            return g
    return 1


_NC_CACHE = {}


def _build_nc(nt: int, zb: bool = True):
    """Build the per-core Bass graph for nt tiles of 512 pairs.
    zb: zero-bias fast path (b1 == 0) merges the two y1 evacuations."""
    from concourse import bacc, mybir, tile

    f32 = mybir.dt.float32
    bf16 = mybir.dt.bfloat16
    fp8e3 = mybir.dt.float8e3
    RELU = mybir.ActivationFunctionType.Relu
    ADD = mybir.AluOpType.add
    MAX = mybir.AluOpType.max

    GT_ = _pick_gt(nt)
    assert nt % GT_ == 0
    ng = nt // GT_

    nc = bacc.Bacc("TRN2", target_bir_lowering=False, debug=False)

    # x in fp8 e3m4, group-major DRAM layout: group g's [128, GT*1024]
    # block is a contiguous DRAM region (large DMA descriptors)
    x_e = nc.declare_dram_parameter("xin", [ng * 128, GT_ * 2 * TILE],
                                    fp8e3, isOutput=False)
    w1_e = nc.declare_dram_parameter("w1", [FEAT, H1], bf16, isOutput=False)
    w2_e = nc.declare_dram_parameter("w2", [H1, H2], bf16, isOutput=False)
    w3_e = nc.declare_dram_parameter("w3", [H2, H3], bf16, isOutput=False)
    b1_e = nc.declare_dram_parameter("b1", [128, 2], f32, isOutput=False)
    b2_e = nc.declare_dram_parameter("b2", [128, 1], f32, isOutput=False)
    b3_e = nc.declare_dram_parameter("b3", [128, 1], f32, isOutput=False)
    # t3 output, tile-major: tile t -> rows [t*128, (t+1)*128), so each
    # per-tile DMA writes one contiguous 128KB DRAM block
    out_e = nc.declare_dram_parameter("t3out", [nt * 128, TILE], bf16,
                                      isOutput=True)

    with tile.TileContext(nc) as tc:
        with (
            tc.tile_pool(name="const", bufs=1) as cpool,
            tc.tile_pool(name="xpool", bufs=3) as xpool,
            tc.tile_pool(name="act", bufs=3) as apool,
            tc.tile_pool(name="ps_y1a", bufs=1, space="PSUM") as ps_y1a,
            tc.tile_pool(name="ps_y1b", bufs=1, space="PSUM") as ps_y1b,
            tc.tile_pool(name="ps_y2", bufs=1, space="PSUM") as ps_y2,
            tc.tile_pool(name="ps_y3", bufs=2, space="PSUM") as ps_y3,
        ):
            # prefetch the first x superblocks before the constants
            xs_tiles = {}

            def prefetch(g):
                if g in xs_tiles or g >= ng:
                    return
                xs = xpool.tile([128, GT_ * 2 * TILE], fp8e3, tag="xs",
                                name="xs")
                # per-tile DMAs so the first L1 of the group can start
                # after 1/GT of the group's data has landed
                for tt in range(GT_):
                    nc.sync.dma_start(
                        out=xs[:, tt * 1024:(tt + 1) * 1024],
                        in_=x_e[g * 128:(g + 1) * 128,
                                tt * 1024:(tt + 1) * 1024])
                xs_tiles[g] = xs

            # w1 first: it is tiny and gates the first matmul
            w1s = cpool.tile([FEAT, H1], bf16, tag="w1s")
            nc.sync.dma_start(out=w1s[:], in_=w1_e[:, :])
            b1s = cpool.tile([128, 2], f32, tag="b1s")
            nc.sync.dma_start(out=b1s[:], in_=b1_e[:, :])
            prefetch(0)
            prefetch(1)
            w2s0 = cpool.tile([128, H2], bf16, tag="w2s0")
            nc.sync.dma_start(out=w2s0[:], in_=w2_e[0:128, :])
            w2s1 = cpool.tile([128, H2], bf16, tag="w2s1")
            nc.sync.dma_start(out=w2s1[:], in_=w2_e[128:256, :])
            w3s = cpool.tile([H2, H3], bf16, tag="w3s")
            nc.sync.dma_start(out=w3s[:], in_=w3_e[:, :])
            b2s = cpool.tile([128, 1], f32, tag="b2s")
            nc.sync.dma_start(out=b2s[:], in_=b2_e[:, :])
            b3s = cpool.tile([128, 1], f32, tag="b3s")
            nc.sync.dma_start(out=b3s[:], in_=b3_e[:, :])

            y1s_t, y2s_t = {}, {}

            def st_l1(i):
                """L1 of tile i.  y1 feat-half 0 evacuates on ACT as soon
                as its two matmuls are done (mid-L1); feat-half 1 on DVE.
                Keeping the two halves in separate 2-bank psum tiles lets
                both evacuations run in parallel, so tile i+1's L1 never
                stalls on psum reuse."""
                g, tt = divmod(i, GT_)
                if tt == 0:
                    prefetch(g)
                    prefetch(g + 1)
                    xs_tiles.pop(g - 2, None)
                xab = xs_tiles[g][:, tt * 1024:(tt + 1) * 1024]  # [AB | BA]

                y1s = apool.tile([128, 2048], bf16, tag="y1s",
                                 name="y1s")
                y1a = ps_y1a.tile([128, 1024], f32, tag="y1a", name="y1a")
                for h in (0, 1):
                    nc.tensor.matmul(
                        out=y1a[:, h * 512:(h + 1) * 512],
                        lhsT=w1s[:, 0:128],
                        rhs=xab[:, h * 512:(h + 1) * 512],
                        start=True, stop=True, skip_group_check=True)
                nc.scalar.activation(out=y1s[:, 0:1024], in_=y1a[:],
                                     func=RELU, bias=b1s[:, 0:1])

                y1b = ps_y1b.tile([128, 1024], f32, tag="y1b", name="y1b")
                for h in (0, 1):
                    nc.tensor.matmul(
                        out=y1b[:, h * 512:(h + 1) * 512],
                        lhsT=w1s[:, 128:256],
                        rhs=xab[:, h * 512:(h + 1) * 512],
                        start=True, stop=True, skip_group_check=True)
                nc.vector.tensor_scalar(out=y1s[:, 1024:2048],
                                        in0=y1b[:],
                                        scalar1=b1s[:, 1:2], scalar2=0.0,
                                        op0=ADD, op1=MAX)
                y1s_t[i] = y1s

            def st_l2(t):
                """L2 of tile t (K=256 via 2 accumulating matmuls) + y2
                evacuation (DVE)."""
                y1s = y1s_t.pop(t)
                y2p = ps_y2.tile([128, 1024], f32, tag="y2p", name="y2p")
                for s in (0, 1):
                    w2c = w2s0 if s == 0 else w2s1
                    for h in (0, 1):
                        nc.tensor.matmul(
                            out=y2p[:, h * 512:(h + 1) * 512],
                            lhsT=w2c[:],
                            rhs=y1s[:, s * 1024 + h * 512:
                                    s * 1024 + (h + 1) * 512],
                            start=(s == 0), stop=(s == 1),
                            skip_group_check=True)
                y2s = apool.tile([128, 1024], bf16, tag="y2s", name="y2s")
                nc.scalar.activation(out=y2s[:], in_=y2p[:],
                                     func=RELU, bias=b2s[:, 0:1])
                y2s_t[t] = y2s

            def st_l3(t):
                """L3 of tile t (AB -> psum partitions 0-63, BA -> 64-127)
                + t3 evacuation (DVE) + DMA to DRAM."""
                y2s = y2s_t.pop(t)
                y3p = ps_y3.tile([128, 512], f32, tag="y3p", name="y3p")
                nc.tensor.matmul(out=y3p[0:64, :], lhsT=w3s[:],
                                 rhs=y2s[:, 0:512], start=True, stop=True,
                                 skip_group_check=True, tile_position=(0, 0))
                nc.tensor.matmul(out=y3p[64:128, :], lhsT=w3s[:],
                                 rhs=y2s[:, 512:1024], start=True,
                                 stop=True, skip_group_check=True,
                                 tile_position=(0, 64))
                t3 = apool.tile([128, 512], bf16, tag="t3", name="t3")
                nc.vector.tensor_scalar(out=t3[:], in0=y3p[:],
                                        scalar1=b3s[:, 0:1], scalar2=0.0,
                                        op0=ADD, op1=MAX)
                nc.sync.dma_start(
                    out=out_e[t * 128:(t + 1) * 128, :], in_=t3[:])

            for i in range(nt + 2):
                if i < nt:
                    st_l1(i)
                if 1 <= i < nt + 1:
                    st_l2(i - 1)
                if i >= 2:
                    st_l3(i - 2)

    nc.finalize()
    return nc


LAST_ZB = True


def _get_nc(nt: int, jw=None, gt=None):
    # jw/gt accepted for test.py compatibility; ignored in v4
    key = (nt, LAST_ZB)
    if key not in _NC_CACHE:
        _NC_CACHE[key] = _build_nc(nt, LAST_ZB)
    return _NC_CACHE[key]


class _Runner:
    """Reusable SPMD executor for a built Bass graph (mirrors
    bass2jax.run_bass_via_pjrt's multi-core path, but keeps the jitted
    callable and device-resident inputs so executions can be repeated and
    timed)."""

    def __init__(self, nc, ncores):
        import jax
        from jax.sharding import Mesh, PartitionSpec, NamedSharding
        from jax.experimental.shard_map import shard_map
        from concourse import bass2jax, mybir

        bass2jax.install_neuronx_cc_hook()
        self.ncores = ncores
        partition_name = (nc.partition_id_tensor.name
                          if nc.partition_id_tensor else None)
        in_names, out_names, out_avals, zero_outs = [], [], [], []
        for alloc in nc.m.functions[0].allocations:
            if not isinstance(alloc, mybir.MemoryLocationSet):
                continue
            name = alloc.memorylocations[0].name
            if alloc.kind == "ExternalInput":
                if name != partition_name:
                    in_names.append(name)
            elif alloc.kind == "ExternalOutput":
                out_names.append(name)
                shape = tuple(alloc.tensor_shape)
                dtype = mybir.dt.np(alloc.dtype)
                out_avals.append(jax.core.ShapedArray(shape, dtype))
                zero_outs.append(np.zeros((ncores * shape[0], *shape[1:]), dtype))
        self.in_names = list(in_names)
        self.out_names = list(out_names)
        self.out_avals = out_avals
        self.zero_outs = zero_outs
        n_params = len(in_names)
        all_in_names = in_names + out_names
        if partition_name is not None:
            all_in_names = all_in_names + [partition_name]

        def _bind(operands):
            if partition_name is not None:
                operands = operands + [bass2jax.partition_id_tensor()]
            return bass2jax._bass_exec_p.bind(
                *operands,
                out_avals=tuple(out_avals),
                in_names=tuple(all_in_names),
                out_names=tuple(out_names),
                lowering_input_output_aliases=(),
                sim_require_finite=True,
                sim_require_nnan=True,
                nc=nc,
            )

        def _make_fn(k):
            def _body(*args):
                ins = list(args[:n_params])
                zouts = tuple(args[n_params:])
                if k == 1:
                    return tuple(_bind(ins + list(zouts)))

                # repeat the NEFF k times in one dispatch via lax.scan,
                # chaining outputs into the next rep's out-buffers (the
                # compile hook allows only one bass_exec per HLO module)
                def step(carry, _):
                    return tuple(_bind(ins + list(carry))), ()

                carry, _ = jax.lax.scan(step, zouts, None, length=k)
                return tuple(carry)

            return jax.jit(
                shard_map(_body, mesh=self.mesh, in_specs=in_specs,
                          out_specs=out_specs, check_rep=False),
                donate_argnums=tuple(range(n_params, n_params + len(out_names))),
                keep_unused=True,
            )

        devices = jax.devices()[:ncores]
        self.mesh = Mesh(np.asarray(devices), ("core",))
        self.sharding = NamedSharding(self.mesh, PartitionSpec("core"))
        in_specs = (PartitionSpec("core"),) * (n_params + len(out_names))
        out_specs = (PartitionSpec("core"),) * len(out_names)
        self._make_fn = _make_fn
        self._fns = {}
        self.fn = self._get_fn(1)
        self.dev_in = None

    def _get_fn(self, k):
        if k not in self._fns:
            self._fns[k] = self._make_fn(k)
        return self._fns[k]

    def load_inputs(self, in_maps):
        import jax
        concat = [
            np.concatenate([np.asarray(in_maps[c][k]) for c in range(self.ncores)],
                           axis=0)
            for k in self.in_names
        ]
        self.dev_in = [jax.device_put(a, self.sharding) for a in concat]

    def run(self):
        import jax
        zeros = [np.zeros_like(z) for z in self.zero_outs]
        outs = self.fn(*self.dev_in, *zeros)
        outs = [np.asarray(o) for o in outs]
        return [
            {name: outs[i].reshape(self.ncores, *self.out_avals[i].shape)[c]
             for i, name in enumerate(self.out_names)}
            for c in range(self.ncores)
        ]

    def bench(self, n=5, k=1):
        import time, jax
        fn = self._get_fn(k)
        times = []
        for _ in range(n):
            zeros = [np.zeros_like(z) for z in self.zero_outs]
            t0 = time.perf_counter()
            outs = fn(*self.dev_in, *zeros)
            jax.block_until_ready(outs)
            times.append(time.perf_counter() - t0)
        return times

    def bench_exec_ns(self, n=6, k1=1, k2=17):
        """Per-NEFF-execution time via differential timing: k2 vs k1 chained
        executions inside one dispatch cancels the (large) dispatch overhead."""
        t1 = self.bench(n=n, k=k1)
        t2 = self.bench(n=n, k=k2)
        per = (min(t2) - min(t1)) / (k2 - k1)
        return per * 1e9, t1, t2


_RUNNER_CACHE = {}
LAST_RUNNER = None
LAST_IN_MAPS = None
LAST_NT = None
LAST_JW = 8
LAST_GT = 4


def _get_runner(nt, ncores, zb):
    key = (nt, ncores, zb)
    if key not in _RUNNER_CACHE:
        _RUNNER_CACHE[key] = _Runner(_get_nc(nt), ncores)
    return _RUNNER_CACHE[key]


def _prep_core_inputs(hAB, hBA, weights, lo, hi, pc):
    """Build the in_map for one core covering pairs [lo, hi)."""
    nt = pc // TILE
    gt = _pick_gt(nt)
    ng = nt // gt
    n = hi - lo

    # x: group-major [ng*128, gt*1024], per tile [AB.T | BA.T] in e3m4
    X = np.zeros((nt, 2, TILE, FEAT), dtype=E3)
    buf = np.zeros((nt * TILE, FEAT), dtype=E3)
    buf[:n] = hAB[lo:hi].astype(E3)
    X[:, 0] = buf.reshape(nt, TILE, FEAT)
    buf[:n] = hBA[lo:hi].astype(E3)
    X[:, 1] = buf.reshape(nt, TILE, FEAT)
    xin = np.ascontiguousarray(
        X.transpose(3, 0, 1, 2).reshape(FEAT, ng, gt * 2 * TILE)
        .transpose(1, 0, 2).reshape(ng * 128, gt * 2 * TILE))

    m = {"xin": xin}
    m.update(weights)
    return m


def _run(hAB, hBA, cutoff, dimer_ind, W1, b1, W2, b2, W3, b3, W4, b4,
         pc=PC, ncores=NCORES):
    global LAST_RUNNER, LAST_IN_MAPS, LAST_NT, LAST_ZB

    nt = pc // TILE
    hAB = np.asarray(hAB, dtype=np.float32)
    hBA = np.asarray(hBA, dtype=np.float32)
    cutoff = np.asarray(cutoff, dtype=np.float32).reshape(-1)
    dimer = np.asarray(dimer_ind).astype(np.int64).reshape(-1)
    npairs = hAB.shape[0]

    W1 = np.asarray(W1, np.float32); b1 = np.asarray(b1, np.float32).reshape(-1)
    W2 = np.asarray(W2, np.float32); b2 = np.asarray(b2, np.float32).reshape(-1)
    W3 = np.asarray(W3, np.float32); b3 = np.asarray(b3, np.float32).reshape(-1)
    W4 = np.asarray(W4, np.float32).reshape(H3)
    b4 = np.asarray(b4, np.float32).reshape(-1)

    zb = not b1.any()
    LAST_ZB = zb
    weights = {
        "w1": np.ascontiguousarray(W1.astype(BF)),
        "w2": np.ascontiguousarray(W2.astype(BF)),
        "w3": np.ascontiguousarray(W3.astype(BF)),
        "b1": np.ascontiguousarray(b1.reshape(2, 128).T.astype(np.float32)),
        "b2": np.ascontiguousarray(b2.reshape(128, 1)),
        "b3": np.ascontiguousarray(np.tile(b3, 2).reshape(128, 1)),
    }

    in_maps = []
    for c in range(ncores):
        lo = min(c * pc, npairs)
        hi = min((c + 1) * pc, npairs)
        in_maps.append(_prep_core_inputs(hAB, hBA, weights, lo, hi, pc))

    runner = _get_runner(nt, ncores, zb)
    runner.load_inputs(in_maps)
    LAST_RUNNER = runner
    LAST_IN_MAPS = in_maps
    LAST_NT = nt
    results = runner.run()

    # host L4 + segment reduce in fp32
    w4stack = np.concatenate([W4, W4]).astype(np.float32)  # [128]
    out = np.zeros((NDIMER,), dtype=np.float32)
    for c in range(ncores):
        lo = min(c * pc, npairs)
        hi = min((c + 1) * pc, npairs)
        if hi <= lo:
            continue
        nt_c = pc // TILE
        T = np.asarray(results[c]["t3out"]).astype(np.float32)
        T = T.reshape(nt_c, 128, TILE)                           # tile-major
        E = np.tensordot(w4stack, T, axes=(0, 1)).reshape(-1)    # [pc]
        delta = E[:hi - lo] * cutoff[lo:hi]
        out += np.bincount(dimer[lo:hi], weights=delta,
                           minlength=NDIMER).astype(np.float32)
    if float(b4[0]) != 0.0:
        out += 2.0 * float(b4[0]) * np.bincount(
            dimer, weights=cutoff, minlength=NDIMER).astype(np.float32)
    return out.reshape(NDIMER, 1)


def kernel(**inputs):
    return _run(
        inputs["hAB"], inputs["hBA"], inputs["cutoff"], inputs["dimer_ind"],
        inputs["W1"], inputs["b1"], inputs["W2"], inputs["b2"],
        inputs["W3"], inputs["b3"], inputs["W4"], inputs["b4"],
    )


# revision 15
# speedup vs baseline: 1.1604x; 1.1604x over previous
"""Trainium2 Bass kernel for APNet2 dAPNet2 MPNN readout + segment reduce.

Computes, for 500k atom pairs:
    E = MLP(hAB) + MLP(hBA)          (4-layer MLP, shared weights)
    delta_E = E * cutoff
    out = segment_sum(delta_E, dimer_ind, ndimer=2048)   -> [2048, 1]

Strategy v4 (8 NeuronCores, data-parallel over pairs):
  - Device computes only L1 -> L2 -> L3 (bf16 matmuls, fp32 psum) and
    streams the post-relu L3 activations (t3, bf16) straight to DRAM via
    DMA.  The tiny L4 (64->1 dot) and the segment reduce run on the HOST
    in fp32: they are ~0.2% of the FLOPs but on-device they cost a
    feature->pair transpose (one stationary load per 128 pairs) plus two
    extra PSUM evacuations, and the PSUM evacuation engines (ACT 1.2GHz
    + DVE 0.96GHz, 1 elem/lane/cycle, the only engines that can read
    PSUM) are co-bottlenecks of this kernel alongside the PE.
  - This also removes the host-side sort + one-hot segment machinery of
    the previous version: the host just does E = w4.T @ t3 and one
    np.bincount scatter over the unsorted pairs.
  - fp8 DoubleRow matmuls were evaluated and rejected: walrus's ISA
    check (s3d3_mm_valid_dst_partition) forbids DoubleRow outputs on
    partitions >= 64, so every DR output collapses onto partitions 0-63
    and all downstream activations become 64-partition tensors, doubling
    their PSUM-evacuation free-size.  The evacuation engines, not the
    PE, set the floor, so 2x matmul rate is a net loss.
  - Per 512-pair tile (1024 MLP rows): PE = 2048 (L1) + 2048 (L2) +
    1024 (L3) cycles @2.4GHz = 2.13us; ACT = y1 evac (2048 lanecycles);
    DVE = y2 + t3 evac (1536 lanecycles); DMA = 256KB x in + 128KB t3
    out.  All engines land just around ~2us/tile.
"""

import numpy as np
import ml_dtypes

BF = ml_dtypes.bfloat16
E3 = ml_dtypes.float8_e3m4

NCORES = 8
N_PAIRS = 500_000
FEAT = 128
NDIMER = 2048
H1, H2, H3 = 256, 128, 64
TILE = 512          # pairs per tile

# per-core pair count (padded)
PC = 62976          # = 512 * 123 ; 8 * 62976 = 503808 >= 500000
NT = PC // TILE     # 123


def _pick_gt(nt):
    for g in (4, 3, 2):
        if nt % g == 0:
            return g
    return 1


_NC_CACHE = {}


def _build_nc(nt: int, zb: bool = True):
    """Build the per-core Bass graph for nt tiles of 512 pairs.
    zb: zero-bias fast path (b1 == 0) merges the two y1 evacuations."""
    from concourse import bacc, mybir, tile

    f32 = mybir.dt.float32
    bf16 = mybir.dt.bfloat16
    fp8e3 = mybir.dt.float8e3
    RELU = mybir.ActivationFunctionType.Relu
    ADD = mybir.AluOpType.add
    MAX = mybir.AluOpType.max

    GT_ = _pick_gt(nt)
    assert nt % GT_ == 0
    ng = nt // GT_

    nc = bacc.Bacc("TRN2", target_bir_lowering=False, debug=False)

    # x in fp8 e3m4, group-major DRAM layout: group g's [128, GT*1024]
    # block is a contiguous DRAM region (large DMA descriptors)
    x_e = nc.declare_dram_parameter("xin", [ng * 128, GT_ * 2 * TILE],
                                    fp8e3, isOutput=False)
    w1_e = nc.declare_dram_parameter("w1", [FEAT, H1], bf16, isOutput=False)
    w2_e = nc.declare_dram_parameter("w2", [H1, H2], bf16, isOutput=False)
    w3_e = nc.declare_dram_parameter("w3", [H2, H3], bf16, isOutput=False)
    b1_e = nc.declare_dram_parameter("b1", [128, 2], f32, isOutput=False)
    b2_e = nc.declare_dram_parameter("b2", [128, 1], f32, isOutput=False)
    b3_e = nc.declare_dram_parameter("b3", [128, 1], f32, isOutput=False)
    # t3 output, tile-major: tile t -> rows [t*128, (t+1)*128), so each
    # per-tile DMA writes one contiguous 128KB DRAM block
    out_e = nc.declare_dram_parameter("t3out", [nt * 128, TILE], bf16,
                                      isOutput=True)

    with tile.TileContext(nc) as tc:
        with (
            tc.tile_pool(name="const", bufs=1) as cpool,
            tc.tile_pool(name="xpool", bufs=3) as xpool,
            tc.tile_pool(name="act", bufs=2) as apool,
            tc.tile_pool(name="ps_y1a", bufs=1, space="PSUM") as ps_y1a,
            tc.tile_pool(name="ps_y1b", bufs=1, space="PSUM") as ps_y1b,
            tc.tile_pool(name="ps_y2", bufs=1, space="PSUM") as ps_y2,
            tc.tile_pool(name="ps_y3", bufs=2, space="PSUM") as ps_y3,
        ):
            # prefetch the first x superblocks before the constants
            xs_tiles = {}

            def prefetch(g):
                if g in xs_tiles or g >= ng:
                    return
                xs = xpool.tile([128, GT_ * 2 * TILE], fp8e3, tag="xs",
                                name="xs")
                # per-tile DMAs so the first L1 of the group can start
                # after 1/GT of the group's data has landed
                for tt in range(GT_):
                    nc.sync.dma_start(
                        out=xs[:, tt * 1024:(tt + 1) * 1024],
                        in_=x_e[g * 128:(g + 1) * 128,
                                tt * 1024:(tt + 1) * 1024])
                xs_tiles[g] = xs

            # w1 first: it is tiny and gates the first matmul
            w1s = cpool.tile([FEAT, H1], bf16, tag="w1s")
            nc.sync.dma_start(out=w1s[:], in_=w1_e[:, :])
            b1s = cpool.tile([128, 2], f32, tag="b1s")
            nc.sync.dma_start(out=b1s[:], in_=b1_e[:, :])
            prefetch(0)
            prefetch(1)
            w2s0 = cpool.tile([128, H2], bf16, tag="w2s0")
            nc.sync.dma_start(out=w2s0[:], in_=w2_e[0:128, :])
            w2s1 = cpool.tile([128, H2], bf16, tag="w2s1")
            nc.sync.dma_start(out=w2s1[:], in_=w2_e[128:256, :])
            w3s = cpool.tile([H2, H3], bf16, tag="w3s")
            nc.sync.dma_start(out=w3s[:], in_=w3_e[:, :])
            b2s = cpool.tile([128, 1], f32, tag="b2s")
            nc.sync.dma_start(out=b2s[:], in_=b2_e[:, :])
            b3s = cpool.tile([128, 1], f32, tag="b3s")
            nc.sync.dma_start(out=b3s[:], in_=b3_e[:, :])

            y1s_t, y2s_t = {}, {}

            def st_l1(i):
                """L1 of tile i.  y1 feat-half 0 evacuates on ACT as soon
                as its two matmuls are done (mid-L1); feat-half 1 on DVE.
                Keeping the two halves in separate 2-bank psum tiles lets
                both evacuations run in parallel, so tile i+1's L1 never
                stalls on psum reuse."""
                g, tt = divmod(i, GT_)
                if tt == 0:
                    prefetch(g)
                    prefetch(g + 1)
                    xs_tiles.pop(g - 2, None)
                xab = xs_tiles[g][:, tt * 1024:(tt + 1) * 1024]  # [AB | BA]

                y1s = apool.tile([128, 2048], bf16, tag="y1s",
                                 name="y1s")
                y1a = ps_y1a.tile([128, 1024], f32, tag="y1a", name="y1a")
                for h in (0, 1):
                    nc.tensor.matmul(
                        out=y1a[:, h * 512:(h + 1) * 512],
                        lhsT=w1s[:, 0:128],
                        rhs=xab[:, h * 512:(h + 1) * 512],
                        start=True, stop=True, skip_group_check=True)
                nc.scalar.activation(out=y1s[:, 0:1024], in_=y1a[:],
                                     func=RELU, bias=b1s[:, 0:1])

                y1b = ps_y1b.tile([128, 1024], f32, tag="y1b", name="y1b")
                for h in (0, 1):
                    nc.tensor.matmul(
                        out=y1b[:, h * 512:(h + 1) * 512],
                        lhsT=w1s[:, 128:256],
                        rhs=xab[:, h * 512:(h + 1) * 512],
                        start=True, stop=True, skip_group_check=True)
                nc.vector.tensor_scalar(out=y1s[:, 1024:2048],
                                        in0=y1b[:],
                                        scalar1=b1s[:, 1:2], scalar2=0.0,
                                        op0=ADD, op1=MAX)
                y1s_t[i] = y1s

            def st_l2(t):
                """L2 of tile t (K=256 via 2 accumulating matmuls) + y2
                evacuation (DVE)."""
                y1s = y1s_t.pop(t)
                y2p = ps_y2.tile([128, 1024], f32, tag="y2p", name="y2p")
                for s in (0, 1):
                    w2c = w2s0 if s == 0 else w2s1
                    for h in (0, 1):
                        nc.tensor.matmul(
                            out=y2p[:, h * 512:(h + 1) * 512],
                            lhsT=w2c[:],
                            rhs=y1s[:, s * 1024 + h * 512:
                                    s * 1024 + (h + 1) * 512],
                            start=(s == 0), stop=(s == 1),
                            skip_group_check=True)
                y2s = apool.tile([128, 1024], bf16, tag="y2s", name="y2s")
                nc.scalar.activation(out=y2s[:], in_=y2p[:],
                                     func=RELU, bias=b2s[:, 0:1])
                y2s_t[t] = y2s

            def st_l3(t):
                """L3 of tile t (AB -> psum partitions 0-63, BA -> 64-127)
                + t3 evacuation (DVE) + DMA to DRAM."""
                y2s = y2s_t.pop(t)
                y3p = ps_y3.tile([128, 512], f32, tag="y3p", name="y3p")
                nc.tensor.matmul(out=y3p[0:64, :], lhsT=w3s[:],
                                 rhs=y2s[:, 0:512], start=True, stop=True,
                                 skip_group_check=True, tile_position=(0, 0))
                nc.tensor.matmul(out=y3p[64:128, :], lhsT=w3s[:],
                                 rhs=y2s[:, 512:1024], start=True,
                                 stop=True, skip_group_check=True,
                                 tile_position=(0, 64))
                t3 = apool.tile([128, 512], bf16, tag="t3", name="t3")
                nc.vector.tensor_scalar(out=t3[:], in0=y3p[:],
                                        scalar1=b3s[:, 0:1], scalar2=0.0,
                                        op0=ADD, op1=MAX)
                nc.sync.dma_start(
                    out=out_e[t * 128:(t + 1) * 128, :], in_=t3[:])

            for i in range(nt + 2):
                if i < nt:
                    st_l1(i)
                if 1 <= i < nt + 1:
                    st_l2(i - 1)
                if i >= 2:
                    st_l3(i - 2)

    nc.finalize()
    return nc


LAST_ZB = True


def _get_nc(nt: int, jw=None, gt=None):
    # jw/gt accepted for test.py compatibility; ignored in v4
    key = (nt, LAST_ZB)
    if key not in _NC_CACHE:
        _NC_CACHE[key] = _build_nc(nt, LAST_ZB)
    return _NC_CACHE[key]


class _Runner:
    """Reusable SPMD executor for a built Bass graph (mirrors
    bass2jax.run_bass_via_pjrt's multi-core path, but keeps the jitted
    callable and device-resident inputs so executions can be repeated and
    timed)."""

    def __init__(self, nc, ncores):
        import jax
        from jax.sharding import Mesh, PartitionSpec, NamedSharding
        from jax.experimental.shard_map import shard_map
        from concourse import bass2jax, mybir

        bass2jax.install_neuronx_cc_hook()
        self.ncores = ncores
        partition_name = (nc.partition_id_tensor.name
                          if nc.partition_id_tensor else None)
        in_names, out_names, out_avals, zero_outs = [], [], [], []
        for alloc in nc.m.functions[0].allocations:
            if not isinstance(alloc, mybir.MemoryLocationSet):
                continue
            name = alloc.memorylocations[0].name
            if alloc.kind == "ExternalInput":
                if name != partition_name:
                    in_names.append(name)
            elif alloc.kind == "ExternalOutput":
                out_names.append(name)
                shape = tuple(alloc.tensor_shape)
                dtype = mybir.dt.np(alloc.dtype)
                out_avals.append(jax.core.ShapedArray(shape, dtype))
                zero_outs.append(np.zeros((ncores * shape[0], *shape[1:]), dtype))
        self.in_names = list(in_names)
        self.out_names = list(out_names)
        self.out_avals = out_avals
        self.zero_outs = zero_outs
        n_params = len(in_names)
        all_in_names = in_names + out_names
        if partition_name is not None:
            all_in_names = all_in_names + [partition_name]

        def _bind(operands):
            if partition_name is not None:
                operands = operands + [bass2jax.partition_id_tensor()]
            return bass2jax._bass_exec_p.bind(
                *operands,
                out_avals=tuple(out_avals),
                in_names=tuple(all_in_names),
                out_names=tuple(out_names),
                lowering_input_output_aliases=(),
                sim_require_finite=True,
                sim_require_nnan=True,
                nc=nc,
            )

        def _make_fn(k):
            def _body(*args):
                ins = list(args[:n_params])
                zouts = tuple(args[n_params:])
                if k == 1:
                    return tuple(_bind(ins + list(zouts)))

                # repeat the NEFF k times in one dispatch via lax.scan,
                # chaining outputs into the next rep's out-buffers (the
                # compile hook allows only one bass_exec per HLO module)
                def step(carry, _):
                    return tuple(_bind(ins + list(carry))), ()

                carry, _ = jax.lax.scan(step, zouts, None, length=k)
                return tuple(carry)

            return jax.jit(
                shard_map(_body, mesh=self.mesh, in_specs=in_specs,
                          out_specs=out_specs, check_rep=False),
                donate_argnums=tuple(range(n_params, n_params + len(out_names))),
                keep_unused=True,
            )

        devices = jax.devices()[:ncores]
        self.mesh = Mesh(np.asarray(devices), ("core",))
        self.sharding = NamedSharding(self.mesh, PartitionSpec("core"))
        in_specs = (PartitionSpec("core"),) * (n_params + len(out_names))
        out_specs = (PartitionSpec("core"),) * len(out_names)
        self._make_fn = _make_fn
        self._fns = {}
        self.fn = self._get_fn(1)
        self.dev_in = None

    def _get_fn(self, k):
        if k not in self._fns:
            self._fns[k] = self._make_fn(k)
        return self._fns[k]

    def load_inputs(self, in_maps):
        import jax
        concat = [
            np.concatenate([np.asarray(in_maps[c][k]) for c in range(self.ncores)],
                           axis=0)
            for k in self.in_names
        ]
        self.dev_in = [jax.device_put(a, self.sharding) for a in concat]

    def run(self):
        import jax
        zeros = [np.zeros_like(z) for z in self.zero_outs]
        outs = self.fn(*self.dev_in, *zeros)
        outs = [np.asarray(o) for o in outs]
        return [
            {name: outs[i].reshape(self.ncores, *self.out_avals[i].shape)[c]
             for i, name in enumerate(self.out_names)}
            for c in range(self.ncores)
        ]

    def bench(self, n=5, k=1):
        import time, jax
        fn = self._get_fn(k)
        times = []
        for _ in range(n):
            zeros = [np.zeros_like(z) for z in self.zero_outs]
            t0 = time.perf_counter()
            outs = fn(*self.dev_in, *zeros)
            jax.block_until_ready(outs)
            times.append(time.perf_counter() - t0)
        return times

    def bench_exec_ns(self, n=6, k1=1, k2=17):
        """Per-NEFF-execution time via differential timing: k2 vs k1 chained
        executions inside one dispatch cancels the (large) dispatch overhead."""
        t1 = self.bench(n=n, k=k1)
        t2 = self.bench(n=n, k=k2)
        per = (min(t2) - min(t1)) / (k2 - k1)
        return per * 1e9, t1, t2


_RUNNER_CACHE = {}
LAST_RUNNER = None
LAST_IN_MAPS = None
LAST_NT = None
LAST_JW = 8
LAST_GT = 4


def _get_runner(nt, ncores, zb):
    key = (nt, ncores, zb)
    if key not in _RUNNER_CACHE:
        _RUNNER_CACHE[key] = _Runner(_get_nc(nt), ncores)
    return _RUNNER_CACHE[key]


def _prep_core_inputs(hAB, hBA, weights, lo, hi, pc):
    """Build the in_map for one core covering pairs [lo, hi)."""
    nt = pc // TILE
    gt = _pick_gt(nt)
    ng = nt // gt
    n = hi - lo

    # x: group-major [ng*128, gt*1024], per tile [AB.T | BA.T] in e3m4
    X = np.zeros((nt, 2, TILE, FEAT), dtype=E3)
    buf = np.zeros((nt * TILE, FEAT), dtype=E3)
    buf[:n] = hAB[lo:hi].astype(E3)
    X[:, 0] = buf.reshape(nt, TILE, FEAT)
    buf[:n] = hBA[lo:hi].astype(E3)
    X[:, 1] = buf.reshape(nt, TILE, FEAT)
    xin = np.ascontiguousarray(
        X.transpose(3, 0, 1, 2).reshape(FEAT, ng, gt * 2 * TILE)
        .transpose(1, 0, 2).reshape(ng * 128, gt * 2 * TILE))

    m = {"xin": xin}
    m.update(weights)
    return m


def _run(hAB, hBA, cutoff, dimer_ind, W1, b1, W2, b2, W3, b3, W4, b4,
         pc=PC, ncores=NCORES):
    global LAST_RUNNER, LAST_IN_MAPS, LAST_NT, LAST_ZB

    nt = pc // TILE
    hAB = np.asarray(hAB, dtype=np.float32)
    hBA = np.asarray(hBA, dtype=np.float32)
    cutoff = np.asarray(cutoff, dtype=np.float32).reshape(-1)
    dimer = np.asarray(dimer_ind).astype(np.int64).reshape(-1)
    npairs = hAB.shape[0]

    W1 = np.asarray(W1, np.float32); b1 = np.asarray(b1, np.float32).reshape(-1)
    W2 = np.asarray(W2, np.float32); b2 = np.asarray(b2, np.float32).reshape(-1)
    W3 = np.asarray(W3, np.float32); b3 = np.asarray(b3, np.float32).reshape(-1)
    W4 = np.asarray(W4, np.float32).reshape(H3)
    b4 = np.asarray(b4, np.float32).reshape(-1)

    zb = not b1.any()
    LAST_ZB = zb
    weights = {
        "w1": np.ascontiguousarray(W1.astype(BF)),
        "w2": np.ascontiguousarray(W2.astype(BF)),
        "w3": np.ascontiguousarray(W3.astype(BF)),
        "b1": np.ascontiguousarray(b1.reshape(2, 128).T.astype(np.float32)),
        "b2": np.ascontiguousarray(b2.reshape(128, 1)),
        "b3": np.ascontiguousarray(np.tile(b3, 2).reshape(128, 1)),
    }

    in_maps = []
    for c in range(ncores):
        lo = min(c * pc, npairs)
        hi = min((c + 1) * pc, npairs)
        in_maps.append(_prep_core_inputs(hAB, hBA, weights, lo, hi, pc))

    runner = _get_runner(nt, ncores, zb)
    runner.load_inputs(in_maps)
    LAST_RUNNER = runner
    LAST_IN_MAPS = in_maps
    LAST_NT = nt
    results = runner.run()

    # host L4 + segment reduce in fp32
    w4stack = np.concatenate([W4, W4]).astype(np.float32)  # [128]
    out = np.zeros((NDIMER,), dtype=np.float32)
    for c in range(ncores):
        lo = min(c * pc, npairs)
        hi = min((c + 1) * pc, npairs)
        if hi <= lo:
            continue
        nt_c = pc // TILE
        T = np.asarray(results[c]["t3out"]).astype(np.float32)
        T = T.reshape(nt_c, 128, TILE)                           # tile-major
        E = np.tensordot(w4stack, T, axes=(0, 1)).reshape(-1)    # [pc]
        delta = E[:hi - lo] * cutoff[lo:hi]
        out += np.bincount(dimer[lo:hi], weights=delta,
                           minlength=NDIMER).astype(np.float32)
    if float(b4[0]) != 0.0:
        out += 2.0 * float(b4[0]) * np.bincount(
            dimer, weights=cutoff, minlength=NDIMER).astype(np.float32)
    return out.reshape(NDIMER, 1)


def kernel(**inputs):
    return _run(
        inputs["hAB"], inputs["hBA"], inputs["cutoff"], inputs["dimer_ind"],
        inputs["W1"], inputs["b1"], inputs["W2"], inputs["b2"],
        inputs["W3"], inputs["b3"], inputs["W4"], inputs["b4"],
    )
